# revision 2
# baseline (speedup 1.0000x reference)
"""Co-attention kernel for Trainium2 (8 NeuronCores, data-parallel over batch).

Reference computation (B=32, L1=L2=1024, D=512):
    u  = u_fea @ Wu.T + bu            # (B, L1, D)
    i  = i_fea @ Wi.T + bi            # (B, L2, D)
    S  = (u @ M) @ i.T                # (B, L1, L2)
    u_score = S.max(axis=2); i_score = S.max(axis=1)
    p_u = softmax(u_score, axis=1)[:, :, None]
    p_i = softmax(i_score, axis=1)[:, :, None]

Device formulation (per batch, weights folded on host):
    W2  = Wu.T @ M        (D, D)   b2 = bu @ M   (D,)
    WiT = Wi.T            (D, D)
    uMT[d, l] = sum_e W2[e, d] * u_feaT[e, l] + b2[d]     (= (u @ M).T)
    iT [d, m] = sum_f WiT[f, d] * i_feaT[f, m] + bi[d]    (= i.T)
    S  [l, m] = sum_d uMT[d, l] * iT[d, m]
    u_score[l] = max_m S ; i_score[m] = max_l S

Sharding: batch 32 -> 8 cores x 4 batches. Matmuls in float32r (fp22
multiply, fp32 accumulate). Input transposes on-device via PE transpose.
Softmax on host (negligible FLOPs).
"""

import sys
import numpy as np

if "/opt/trn_rl_repo" not in sys.path:
    sys.path.insert(0, "/opt/trn_rl_repo")

import concourse.bass as bass  # noqa: E402
import concourse.tile as tile  # noqa: E402
import concourse.mybir as mybir  # noqa: E402
from concourse import bacc, bass_isa  # noqa: E402
from concourse.bass import ts  # noqa: E402
from concourse.bass_utils import run_bass_kernel_spmd  # noqa: E402
from concourse.masks import make_identity  # noqa: E402

P = 128
D = 512
L = 1024
NB = 4          # batches per core
NCORES = 8
EC = D // P     # feature chunks (4)
LJ = L // P     # l chunks (8)
F32 = mybir.dt.float32
F32R = mybir.dt.float32r
AF = mybir.ActivationFunctionType

_CACHE = {}


def _build_nc():
    nc = bacc.Bacc("TRN2", target_bir_lowering=False, debug=False,
                   num_devices=NCORES)
    u4 = nc.dram_tensor("u4", [NB, L, D], F32, kind="ExternalInput")
    i4 = nc.dram_tensor("i4", [NB, L, D], F32, kind="ExternalInput")
    w2 = nc.dram_tensor("w2", [D, D], F32R, kind="ExternalInput")
    wit = nc.dram_tensor("wit", [D, D], F32R, kind="ExternalInput")
    b2 = nc.dram_tensor("b2", [D], F32, kind="ExternalInput")
    bi = nc.dram_tensor("bi", [D], F32, kind="ExternalInput")
    us = nc.dram_tensor("us", [NB, L], F32, kind="ExternalOutput")
    isc = nc.dram_tensor("isc", [NB, L], F32, kind="ExternalOutput")

    with tile.TileContext(nc) as tc:
        with (
            tc.tile_pool(name="const", bufs=1) as cpool,
            tc.tile_pool(name="nat", bufs=2) as nat_pool,
            tc.tile_pool(name="feat", bufs=1) as feat_pool,
            tc.tile_pool(name="stg", bufs=1) as stg_pool,
            tc.tile_pool(name="acc", bufs=2) as acc_pool,
            tc.tile_pool(name="psmm", bufs=4, space="PSUM") as psmm,
            tc.tile_pool(name="pstr", bufs=3, space="PSUM") as pstr,
        ):
            ident = cpool.tile([P, P], F32)
            make_identity(nc, ident)
            # weights: [P(e'), EC, D] with e = ec*P + e'
            w2_sb = cpool.tile([P, EC, D], F32R)
            nc.sync.dma_start(w2_sb[:], w2.ap().rearrange("(c p) d -> p c d", p=P))
            wit_sb = cpool.tile([P, EC, D], F32R)
            nc.sync.dma_start(wit_sb[:], wit.ap().rearrange("(c p) d -> p c d", p=P))
            b2_sb = cpool.tile([P, EC], F32)
            nc.sync.dma_start(b2_sb[:], b2.ap().rearrange("(c p) -> p c", p=P))
            bi_sb = cpool.tile([P, EC], F32)
            nc.sync.dma_start(bi_sb[:], bi.ap().rearrange("(c p) -> p c", p=P))

            for b in range(NB):
                # ---- load natural layout [P(l'), LJ, D], l = j*P + l' ----
                u_nat = nat_pool.tile([P, LJ, D], F32, tag="u_nat")
                nc.sync.dma_start(u_nat[:],
                                  u4.ap()[b].rearrange("(j p) e -> p j e", p=P))
                i_nat = nat_pool.tile([P, LJ, D], F32, tag="i_nat")
                nc.sync.dma_start(i_nat[:],
                                  i4.ap()[b].rearrange("(j p) e -> p j e", p=P))

                # ---- transpose to feature-major [P(e'), EC, L] ----
                u_feaT = feat_pool.tile([P, EC, L], F32R, tag="u_feaT")
                i_feaT = feat_pool.tile([P, EC, L], F32R, tag="i_feaT")
                for (nat, feaT, nm) in ((u_nat, u_feaT, "u"), (i_nat, i_feaT, "i")):
                    for j in range(LJ):
                        ps_t = pstr.tile([P, EC, P], F32, tag="ps_t",
                                         name=f"ps_t_{nm}_{b}_{j}")
                        for k in range(EC):
                            nc.tensor.transpose(ps_t[:, k, :],
                                                nat[:, j, ts(k, P)], ident)
                        nc.any.tensor_copy(out=feaT[:, :, ts(j, P)], in_=ps_t[:])

                # ---- stage A: uMT[d, l] = W2^T @ u_feaT + b2 ----
                uMT = stg_pool.tile([P, EC, L], F32R, tag="uMT")
                for dc in range(EC):
                    for lh in range(2):
                        ps_a = psmm.tile([P, 512], F32, tag="ps_mm",
                                         name=f"ps_a_{b}_{dc}_{lh}")
                        for ec in range(EC):
                            nc.tensor.matmul(
                                ps_a[:],
                                w2_sb[:, ec, ts(dc, P)],
                                u_feaT[:, ec, ts(lh, 512)],
                                start=(ec == 0), stop=(ec == EC - 1))
                        nc.scalar.activation(uMT[:, dc, ts(lh, 512)], ps_a[:],
                                             AF.Identity, bias=b2_sb[:, dc:dc + 1])

                # ---- stage B: iT[d, m] = WiT^T @ i_feaT + bi ----
                iT = stg_pool.tile([P, EC, L], F32R, tag="iT")
                for dc in range(EC):
                    for lh in range(2):
                        ps_b = psmm.tile([P, 512], F32, tag="ps_mm",
                                         name=f"ps_b_{b}_{dc}_{lh}")
                        for ec in range(EC):
                            nc.tensor.matmul(
                                ps_b[:],
                                wit_sb[:, ec, ts(dc, P)],
                                i_feaT[:, ec, ts(lh, 512)],
                                start=(ec == 0), stop=(ec == EC - 1))
                        nc.scalar.activation(iT[:, dc, ts(lh, 512)], ps_b[:],
                                             AF.Identity, bias=bi_sb[:, dc:dc + 1])

                # ---- stage C: S[l, m] tiles + reductions ----
                u_parts = acc_pool.tile([P, LJ, 2], F32, tag="u_parts")
                i_acc = acc_pool.tile([P, L], F32, tag="i_acc")
                for j in range(LJ):
                    for mh in range(2):
                        ps_s = psmm.tile([P, 512], F32, tag="ps_mm",
                                         name=f"ps_s_{b}_{j}_{mh}")
                        for dc in range(EC):
                            nc.tensor.matmul(
                                ps_s[:],
                                uMT[:, dc, ts(j, P)],
                                iT[:, dc, ts(mh, 512)],
                                start=(dc == 0), stop=(dc == EC - 1))
                        # row max over m (free dim)
                        nc.vector.reduce_max(u_parts[:, j, mh:mh + 1], ps_s[:],
                                             axis=mybir.AxisListType.X)
                        # col max accumulate over l chunks
                        if j == 0:
                            nc.any.tensor_copy(out=i_acc[:, ts(mh, 512)],
                                               in_=ps_s[:])
                        else:
                            nc.any.tensor_tensor(i_acc[:, ts(mh, 512)], ps_s[:],
                                                 i_acc[:, ts(mh, 512)],
                                                 mybir.AluOpType.max)

                # ---- finals ----
                u_sc = acc_pool.tile([P, LJ], F32, tag="u_sc")
                nc.vector.reduce_max(u_sc[:], u_parts[:],
                                     axis=mybir.AxisListType.X)
                nc.sync.dma_start(us.ap()[b].rearrange("(j p) -> p j", p=P),
                                  u_sc[:])
                i_red = acc_pool.tile([P, L], F32, tag="i_red")
                nc.gpsimd.partition_all_reduce(i_red[:], i_acc[:], channels=P,
                                               reduce_op=bass_isa.ReduceOp.max)
                nc.sync.dma_start(isc.ap()[b].rearrange("(a m) -> a m", a=1),
                                  i_red[0:1, :])

    nc.compile()
    return nc


def _softmax(x):
    x = x.astype(np.float64)
    x = x - x.max(axis=1, keepdims=True)
    e = np.exp(x)
    return (e / e.sum(axis=1, keepdims=True)).astype(np.float32)


def kernel(u_fea, i_fea, M, Wu, bu, Wi, bi):
    u_fea = np.asarray(u_fea, dtype=np.float32)
    i_fea = np.asarray(i_fea, dtype=np.float32)
    M64 = np.asarray(M, dtype=np.float64)
    Wu64 = np.asarray(Wu, dtype=np.float64)
    Wi64 = np.asarray(Wi, dtype=np.float64)
    bu64 = np.asarray(bu, dtype=np.float64)
    bi64 = np.asarray(bi, dtype=np.float64)

    w2 = (Wu64.T @ M64).astype(np.float32)        # (D, D)  [e, d]
    b2 = (bu64 @ M64).astype(np.float32)          # (D,)
    wit = np.ascontiguousarray(Wi64.T).astype(np.float32)  # (D, D) [f, d]
    bi32 = bi64.astype(np.float32)

    if "nc" not in _CACHE:
        _CACHE["nc"] = _build_nc()
    nc = _CACHE["nc"]

    in_maps = []
    for c in range(NCORES):
        in_maps.append({
            "u4": np.ascontiguousarray(u_fea[c * NB:(c + 1) * NB]),
            "i4": np.ascontiguousarray(i_fea[c * NB:(c + 1) * NB]),
            "w2": w2, "wit": wit, "b2": b2, "bi": bi32,
        })
    res = run_bass_kernel_spmd(nc, in_maps, core_ids=list(range(NCORES)))
    _CACHE["last_results"] = res

    u_score = np.concatenate([res.results[c]["us"] for c in range(NCORES)], axis=0)
    i_score = np.concatenate([res.results[c]["isc"] for c in range(NCORES)], axis=0)

    p_u = _softmax(u_score)[:, :, None]
    p_i = _softmax(i_score)[:, :, None]
    return p_u, p_i
